# revision 22
# baseline (speedup 1.0000x reference)
"""Trainium2 Bass kernel for nn_AttentionTail (v2: bf16 streaming).

Reference (B=2, N=300, C=256, H=2 heads, hd=128, L=21760):
  q = query @ Wq.T ; k = key @ Wk.T   (2 heads of 128)
  attn[b,n,l,h] = scale * <q_h, k_h>
  per level i (128^2, 64^2, 32^2, 16^2 keys): z = relu(attn_i @ Wl[i].T + bl[i])
  bilinear-upsample each level map to 128x128, concat channels,
  mask = relu(concat @ Wf.T + bf)

Host folds Wq/Wk/Wl/Wf/scale/signs into 8 per-(level,channel) query vectors
(sign-folded so the device relu is always plain relu; signs ride in the
upsample constants / channel-combine ops).

Device (8 cores SPMD, B x N-quarter, 75 q/core), all bf16 streams:
  keyT bf16 [256, 22016] ordered [lvl1 (w,h) | lvl2 (w,h) | lvl3 (w,h)
  h-padded | lvl0 (h,w)]; scores via bf16 matmuls into 150-wide psum slots;
  fused relu (ACT) -> level buffers; DVE channel-combine; per-query
  transpose+row-upsample fused into one PE matmul (rhs = UrT); col-upsample
  + level-0 accumulate via PE; final relu (ACT) -> bf16 out [c, (n, r)];
  host transposes to [n, r, c] for free.
"""

import sys
import numpy as np

sys.path.insert(0, "/opt/trn_rl_repo")

import concourse.bass as bass
import concourse.bacc as bacc
import concourse.tile as tile
from concourse import mybir
from concourse.bass_utils import run_bass_kernel_spmd

F32 = mybir.dt.float32
BF16 = mybir.dt.bfloat16

B, N, C = 2, 300, 256
HD = 128
SCALE = HD ** -0.5
HW_LVL = ((128, 128), (64, 64), (32, 32), (16, 16))
LVL_OFF = [0, 16384, 20480, 21504]
NQ = 75
NCORES = 8
L_DEV = 22016            # lvl1 4096 | lvl2 1024 | lvl3 512 (h padded) | lvl0 16384
LV_BASE = [5632, 0, 4096, 5120]   # key offset of each level in keyT order
LVL_H = [128, 64, 32, 32]         # partition rows per 128-key block (lvl3 padded)
LVL_W = [128, 64, 32, 16]

# cpack column layout (bf16 [128, 1840]); UR23 = [urT2; urT3] row-stacked
QH0, QH1, UR1, UR23, UCC, IDB = 0, 600, 1200, 1328, 1456, 1584
WBASE = [0, 0, 64, 96]            # zcat partition base per level


def interp_matrix(src, dst):
    U = np.zeros((dst, src), np.float32)
    s = src / dst
    for d in range(dst):
        x = (d + 0.5) * s - 0.5
        x0 = int(np.floor(x))
        fr = x - x0
        a, b = max(0, min(src - 1, x0)), max(0, min(src - 1, x0 + 1))
        U[d, a] += 1 - fr
        U[d, b] += fr
    return U


def _spans(nblk):
    """Split nblk blocks into psum spans: (nb banks, ns slots/bank) per span."""
    out = []
    left = nblk
    while left > 0:
        take = min(6, left)
        if take == 6:
            out.append((2, 3))
        elif take == 4:
            out.append((2, 2))
        elif take == 2:
            out.append((1, 2))
        elif take == 3:
            out.append((1, 3))
        elif take == 1:
            out.append((1, 1))
        else:  # 5 -> 3 + 2
            out.append((1, 3))
            left -= 3
            continue
        left -= take
    return out


def _build_program(sig01, fuse_relu, reps=1):
    nc = bacc.Bacc("TRN2", target_bir_lowering=False)
    keyT = nc.dram_tensor("keyT", [C, L_DEV], BF16, kind="ExternalInput")
    cpack = nc.dram_tensor("cpack", [128, 1840], BF16, kind="ExternalInput")
    bias_in = nc.dram_tensor("bias_in", [1, 9], F32, kind="ExternalInput")
    out_d = nc.dram_tensor("out", [128, NQ * 128], BF16, kind="ExternalOutput")

    with tile.TileContext(nc) as tc:
        for _ in range(reps):
            _body(nc, tc, keyT, cpack, bias_in, out_d, sig01, fuse_relu)
    nc.compile()
    return nc


def _body(nc, tc, keyT, cpack_d, bias_in, out_d, sig01, fuse_relu):
    from contextlib import ExitStack
    ctx = ExitStack()
    add = mybir.AluOpType.add
    sub = mybir.AluOpType.subtract
    relu = mybir.ActivationFunctionType.Relu
    with ctx:
        consts = ctx.enter_context(tc.tile_pool(name="consts", bufs=2))
        kapool = ctx.enter_context(tc.tile_pool(name="kapool", bufs=2))
        k0pool = ctx.enter_context(tc.tile_pool(name="k0pool", bufs=2))
        upool = ctx.enter_context(tc.tile_pool(name="upool", bufs=1))
        spool = ctx.enter_context(tc.tile_pool(name="spool", bufs=1))
        ypool = ctx.enter_context(tc.tile_pool(name="ypool", bufs=1))
        zpool = ctx.enter_context(tc.tile_pool(name="zpool", bufs=1))
        fpool = ctx.enter_context(tc.tile_pool(name="fpool", bufs=1))
        ps_attn = ctx.enter_context(tc.tile_pool(name="ps_attn", bufs=3, space="PSUM"))
        ps_tail = ctx.enter_context(tc.tile_pool(name="ps_tail", bufs=2, space="PSUM"))

        cp = consts.tile([128, 1840], BF16, name="cp")
        nc.sync.dma_start(out=cp, in_=cpack_d[:, :])
        bias_sb = consts.tile([128, 9], F32, name="bias_sb")
        nc.sync.dma_start(out=bias_sb, in_=bias_in[0:1, :].to_broadcast([128, 9]))

        ka0 = kapool.tile([128, 5632], BF16, tag="ka0", name="ka0")
        ka1 = kapool.tile([128, 5632], BF16, tag="ka1", name="ka1")
        nc.sync.dma_start(out=ka0, in_=keyT[0:128, 0:5632])
        nc.sync.dma_start(out=ka1, in_=keyT[128:256, 0:5632])

        u_l = [None,
               upool.tile([128, 32 * 150], BF16, name="u1"),
               upool.tile([128, 8 * 150], BF16, name="u2"),
               upool.tile([128, 4 * 150], BF16, name="u3")]
        sr1 = spool.tile([64, NQ * 64], BF16, name="sr1")
        srB = spool.tile([64, NQ * 48], BF16, name="srB")  # blockdiag lvl2|lvl3
        ycat = ypool.tile([128, 2 * NQ * 128], BF16, name="ycat")
        zcat = zpool.tile([128, NQ * 128], BF16, name="zcat")
        fin = fpool.tile([128, NQ * 128], BF16, name="fin")

        def score_span(lvl, k0t, k1t, tblk0, gblk0, nb, ns):
            """One psum span: nb*ns blocks starting at block tblk0 of (k0t,k1t),
            gblk0 = block index within the level."""
            nblk = nb * ns
            ps = ps_attn.tile([128, 1024], F32, tag="ps", name="ps")
            qc0 = QH0 + lvl * 150
            qc1 = QH1 + lvl * 150
            for j in range(nblk):
                so = (j // ns) * 512 + (j % ns) * 150
                kcol = (tblk0 + j) * 128
                pslice = ps[:, so:so + 150]
                nc.tensor.matmul(pslice, k0t[:, kcol:kcol + 128],
                                 cp[:, qc0:qc0 + 150], start=True, stop=False)
                nc.tensor.matmul(pslice, k1t[:, kcol:kcol + 128],
                                 cp[:, qc1:qc1 + 150], start=False, stop=True)
            # relu (+bias) -> level dst
            psv = ps.rearrange("p (b x) -> p b x", x=512)
            if lvl == 0:
                # dst ycat[c, ch, n, r]: off = ch*9600 + n*128 + (gblk0 + j)
                yv = ycat.rearrange("p (ch n r) -> p ch n r", ch=2, n=NQ)
                if fuse_relu:
                    src = psv[:, 0:nb, 0:ns * 150].rearrange(
                        "p b (s ch n) -> p b s ch n", s=ns, ch=2)
                    dst = yv[:, :, :, gblk0:gblk0 + nblk].rearrange(
                        "p ch n (b s) -> p b s ch n", b=nb)
                    nc.scalar.activation(dst, src, relu,
                                         bias=bias_sb[:, 0:1], scale=1.0)
                else:
                    for ch in range(2):
                        src = psv[:, 0:nb, 0:ns * 150].rearrange(
                            "p b (s ch n) -> p b s ch n", s=ns, ch=2)[:, :, :, ch]
                        dst = yv[:, ch, :, gblk0:gblk0 + nblk].rearrange(
                            "p n (b s) -> p b s n", b=nb)
                        nc.scalar.activation(dst, src, relu,
                                             bias=bias_sb[:, ch:ch + 1], scale=1.0)
            else:
                # dst u_l [p, (J, ch, n)] packed at J = gblk0 + j
                uv = u_l[lvl].rearrange("p (J x) -> p J x", x=150)
                if fuse_relu:
                    src = psv[:, 0:nb, 0:ns * 150].rearrange(
                        "p b (s x) -> p b s x", s=ns)
                    dst = uv[:, gblk0:gblk0 + nblk, :].rearrange(
                        "p (b s) x -> p b s x", b=nb)
                    nc.scalar.activation(dst, src, relu,
                                         bias=bias_sb[:, 2 * lvl:2 * lvl + 1],
                                         scale=1.0)
                else:
                    for ch in range(2):
                        src = psv[:, 0:nb, 0:ns * 150].rearrange(
                            "p b (s ch n) -> p b s ch n", s=ns, ch=2)[:, :, :, ch]
                        dst = uv[:, gblk0:gblk0 + nblk, :].rearrange(
                            "p (b s) (ch n) -> p b s ch n", b=nb, ch=2)[:, :, :, ch]
                        nc.scalar.activation(dst, src, relu,
                                             bias=bias_sb[:, 2 * lvl + ch:2 * lvl + ch + 1],
                                             scale=1.0)

        # ---- levels 1-3 scores ----
        for lvl in (1, 2, 3):
            nblk_l = {1: 32, 2: 8, 3: 4}[lvl]
            g = 0
            for nb, ns in _spans(nblk_l):
                tb = (LV_BASE[lvl] // 128) + g
                score_span(lvl, ka0, ka1, tb, g, nb, ns)
                g += nb * ns

        # ---- zero the off-diagonal blocks of srB (gpsimd; idle engine) ----
        srBv = srB.rearrange("p (n w) -> p n w", n=NQ, w=48)
        nc.gpsimd.memset(srBv[0:32, :, 32:48], 0.0)
        nc.gpsimd.memset(srBv[32:64, :, 0:32], 0.0)

        # ---- DVE channel-combine -> sr (sign-folded: add/sub) ----
        # dst layouts: lvl1 -> sr1 [64h, (n, 64w)]; lvl2 -> srB[0:32, (n, w 0:32)];
        # lvl3 -> srB[32:64, (n, w 32:48)]
        for lvl, dst_rows, wofs in ((1, (0, 64), 0), (2, (0, 32), 0),
                                    (3, (32, 64), 32)):
            h, w = LVL_H[lvl], LVL_W[lvl]
            ws = 128 // h
            J = w // ws
            op = add if sig01[lvl] > 0 else sub
            uv = u_l[lvl].rearrange("p (J ch n) -> p J ch n", ch=2, n=NQ)
            if lvl == 1:
                base = sr1.rearrange("p (n w) -> p n w", n=NQ, w=64)
            else:
                base = srBv[dst_rows[0]:dst_rows[1], :, wofs:wofs + w]
            wv = base.rearrange("p n (J par) -> p n J par", J=J, par=ws)
            for par in range(ws):
                in0 = uv[par * h:(par + 1) * h, :, 0, :]
                in1 = uv[par * h:(par + 1) * h, :, 1, :]
                dst = wv[:, :, :, par].transpose([0, 2, 1])   # [h, J, n]
                nc.vector.tensor_tensor(dst, in0, in1, op)

        # ---- zT: per-query transpose + row-upsample + channel-fold ----
        ngroups = (NQ + 3) // 4

        def emit_zt_group(gq):
            qn0 = gq * 4
            qcnt = min(4, NQ - qn0)
            zt = ps_tail.tile([128, 512], F32, tag="tl", name="zt")
            for k in range(qcnt):
                n = qn0 + k
                nc.tensor.matmul(
                    zt[0:64, k * 128:(k + 1) * 128],
                    sr1[0:64, n * 64:(n + 1) * 64],
                    cp[0:64, UR1:UR1 + 128],
                    start=True, stop=True)
                nc.tensor.matmul(
                    zt[64:112, k * 128:(k + 1) * 128],
                    srB[0:64, n * 48:(n + 1) * 48],
                    cp[0:64, UR23:UR23 + 128],
                    start=True, stop=True)
            nc.vector.tensor_copy(
                zcat[0:112, qn0 * 128:(qn0 + qcnt) * 128],
                zt[0:112, 0:qcnt * 128])

        # ---- level-0 scores (4 chunks), channel-merge overlapped per chunk ----
        op0 = add if sig01[0] > 0 else sub
        yv4 = ycat.rearrange("p (ch n r) -> p ch n r", ch=2, n=NQ)
        for c in range(4):
            kc0 = k0pool.tile([128, 4096], BF16, tag="kc0", name="kc0")
            kc1 = k0pool.tile([128, 4096], BF16, tag="kc1", name="kc1")
            off = 5632 + c * 4096
            nc.sync.dma_start(out=kc0, in_=keyT[0:128, off:off + 4096])
            nc.sync.dma_start(out=kc1, in_=keyT[128:256, off:off + 4096])
            g = 0
            for nb, ns in _spans(32):
                score_span(0, kc0, kc1, g, c * 32 + g, nb, ns)
                g += nb * ns
            for gq in range(c * 5, min((c + 1) * 5, 19) if c < 3 else 19):
                emit_zt_group(gq)
            # merge this chunk's r-range in place into ycat[:, :9600]
            r0, r1 = c * 32, (c + 1) * 32
            nc.vector.tensor_tensor(yv4[:, 0, :, r0:r1], yv4[:, 0, :, r0:r1],
                                    yv4[:, 1, :, r0:r1], op0)

        # ---- col-upsample + level-0 accumulate + final relu + store ----
        for gq in range(ngroups):
            qn0 = gq * 4
            qcnt = min(4, NQ - qn0)
            cols = qcnt * 128
            pb = ps_tail.tile([128, 512], F32, tag="tl", name="pb")
            for c0 in range(0, cols, 512):
                cw = min(512, cols - c0)
                gc = qn0 * 128 + c0
                nc.tensor.matmul(pb[:, c0:c0 + cw], cp[0:112, UCC:UCC + 128],
                                 zcat[0:112, gc:gc + cw], start=True, stop=False)
                nc.tensor.matmul(pb[:, c0:c0 + cw], cp[:, IDB:IDB + 128],
                                 ycat[:, gc:gc + cw], start=False, stop=True)
            if gq % 2 == 1:   # rebalance ACT vs DVE
                nc.vector.tensor_scalar(fin[:, qn0 * 128:qn0 * 128 + cols],
                                        pb[:, 0:cols], bias_sb[:, 8:9], 0.0,
                                        add, mybir.AluOpType.max)
            else:
                nc.scalar.activation(fin[:, qn0 * 128:qn0 * 128 + cols],
                                     pb[:, 0:cols], relu,
                                     bias=bias_sb[:, 8:9], scale=1.0)
            if gq == 9:
                nc.gpsimd.dma_start(out=out_d[:, 0:5120], in_=fin[:, 0:5120])
            elif gq == ngroups - 1:
                nc.gpsimd.dma_start(out=out_d[:, 5120:NQ * 128],
                                    in_=fin[:, 5120:NQ * 128])


def _host_prep(query, key, Wq, Wk, Wl, bl, Wf, bf):
    import ml_dtypes
    BFNP = ml_dtypes.bfloat16
    query = np.asarray(query, np.float32)
    key = np.asarray(key, np.float32)
    Wq, Wk = np.asarray(Wq, np.float32), np.asarray(Wk, np.float32)
    Wl, bl = np.asarray(Wl, np.float32), np.asarray(bl, np.float32)
    Wf, bf = np.asarray(Wf, np.float32), np.asarray(bf, np.float32)

    qproj = query @ Wq.T
    sig = np.ones((4, 2), np.float32)
    qhat = np.zeros((4, 2, B, N, C), np.float32)
    bhat = np.zeros((4, 2), np.float32)
    for i in range(4):
        for c in range(2):
            wf = float(Wf[0, 2 * i + c])
            s = 1.0 if wf >= 0 else -1.0
            sig[i, c] = s
            qt = np.concatenate([Wl[i][c, 0] * qproj[..., :HD],
                                 Wl[i][c, 1] * qproj[..., HD:]], -1)
            qhat[i, c] = (s * SCALE * wf) * (qt @ Wk)
            bhat[i, c] = s * wf * bl[i][c]
    sig01 = [sig[i, 0] * sig[i, 1] for i in range(4)]
    fuse_relu = bool(np.all(bhat[:, 0] == bhat[:, 1]))

    # keyT: lvl1 (w,h) | lvl2 (w,h) | lvl3 (w,h) h-padded | lvl0 (h,w)
    keyTs = []
    for b in range(B):
        cols = []
        for i in (1, 2):
            h, w = HW_LVL[i]
            blk = key[b, LVL_OFF[i]:LVL_OFF[i] + h * w].reshape(h, w, C)
            cols.append(np.ascontiguousarray(blk.transpose(1, 0, 2)).reshape(-1, C))
        b3 = key[b, LVL_OFF[3]:LVL_OFF[3] + 256].reshape(16, 16, C).transpose(1, 0, 2)
        b3 = np.concatenate([b3, np.zeros((16, 16, C), np.float32)], 1)
        cols.append(b3.reshape(-1, C))
        cols.append(key[b, :16384])
        kb = np.concatenate(cols, 0)
        keyTs.append(np.ascontiguousarray(kb.T).astype(BFNP))

    # upsample consts (sign-folded)
    urT1 = np.zeros((64, 128), np.float32)
    urT1[:, :] = sig[1, 0] * interp_matrix(64, 128).T
    urT23 = np.zeros((64, 128), np.float32)
    urT23[0:32] = sig[2, 0] * interp_matrix(32, 128).T
    urT23[32:48] = sig[3, 0] * interp_matrix(16, 128).T   # rows 48:64 stay 0 (h pad)
    uc = np.zeros((128, 128), np.float32)
    uc[0:64] = interp_matrix(64, 128).T
    uc[64:96] = interp_matrix(32, 128).T
    uc[96:112] = interp_matrix(16, 128).T
    idb = sig[0, 0] * np.eye(128, dtype=np.float32)

    biases = np.zeros((1, 9), np.float32)
    biases[0, 0:8] = bhat.reshape(-1)   # [lvl*2+ch]; slot 0/2/4/6 used when fused
    biases[0, 8] = float(bf[0])

    in_maps = []
    for core in range(NCORES):
        b, q0 = core // 4, (core % 4) * NQ
        cpk = np.zeros((128, 1840), np.float32)
        for i in range(4):
            for c in range(2):
                col = i * 150 + c * 75
                cpk[:, QH0 + col:QH0 + col + 75] = qhat[i, c, b, q0:q0 + NQ, :128].T
                cpk[:, QH1 + col:QH1 + col + 75] = qhat[i, c, b, q0:q0 + NQ, 128:].T
        cpk[0:64, UR1:UR1 + 128] = urT1
        cpk[0:64, UR23:UR23 + 128] = urT23
        cpk[:, UCC:UCC + 128] = uc
        cpk[:, IDB:IDB + 128] = idb
        in_maps.append({
            "keyT": keyTs[b],
            "cpack": cpk.astype(BFNP),
            "bias_in": biases,
        })
    return in_maps, sig01, fuse_relu


def kernel(query, key, Wq, Wk, Wl, bl, Wf, bf, hw_lvl=None, trace=False, reps=1):
    in_maps, sig01, fuse_relu = _host_prep(query, key, Wq, Wk, Wl, bl, Wf, bf)
    nc = _build_program(sig01, fuse_relu, reps=reps)
    res = run_bass_kernel_spmd(nc, in_maps, list(range(NCORES)), trace=trace)
    out = np.zeros((B, N, 128 * 128, 1), np.float32)
    for core in range(NCORES):
        b, q0 = core // 4, (core % 4) * NQ
        arr = np.asarray(res.results[core]["out"]).astype(np.float32)
        out[b, q0:q0 + NQ, :, 0] = arr.reshape(128, NQ, 128).transpose(
            1, 2, 0).reshape(NQ, 128 * 128)
    kernel.last_results = res
    return out


kernel.last_results = None
